# revision 1
# baseline (speedup 1.0000x reference)
"""Trainium2 Bass kernel for nn_Decoder: per-edge bilinear decoder.

  out[e, c] = relu( sum_k (u[e] @ W[k] @ v[e]) * Wc[k, c] )
  u = user_inputs[user_indices], v = item_inputs[item_indices]

Strategy (8 NeuronCores, data-parallel over the edge dim E):
  - Host folds the classifier into the bilinear weights:
      M2[d, c*128+f] = sum_k W[k,d,f] * Wc[k,c]        [128, 640] bf16
    (cuts matmul work K=8 -> C=5), casts both embedding tables to bf16.
  - Each core gets E/8 edges. Edge endpoint rows are gathered on-device with
    dma_gather (int16 indices), so edges are bucketed host-side by
    (user_idx>>15, item_idx>>15) and gathered from sliced table bases;
    outputs are un-permuted on host.
  - Per 128-edge tile: PE transposes u -> uT, one bf16 matmul
    Y[e, 640] = uT.T @ M2 into PSUM, ScalarE copies Y -> SBUF bf16, then 5x
    scalar_tensor_tensor (fused multiply-by-v + free-dim accumulate) on
    VectorE produce basis[e, 5]. Relu on VectorE, batched DMA out.
"""
import sys
import os
import math
import functools

for _p in ("/opt/trn_rl_repo", "/root/.axon_site/_ro/trn_rl_repo"):
    if os.path.isdir(_p) and _p not in sys.path:
        sys.path.insert(0, _p)

import numpy as np
import ml_dtypes

import concourse.bass as bass
import concourse.bacc as bacc
from concourse import mybir
from concourse.tile import TileContext
from concourse.bass_utils import run_bass_kernel_spmd

bf16 = ml_dtypes.bfloat16
F32 = mybir.dt.float32
BF16 = mybir.dt.bfloat16
I16 = mybir.dt.int16

# Problem constants (hardcoded per harness contract)
N_USERS, N_ITEMS, D, E, K, C = 100000, 50000, 128, 1000000, 8, 5
NCORES = 8
EL = E // NCORES              # 125000 edges per core
P = 128
BATCH = 1024                  # edges per gather batch (8 tiles)
TPB = BATCH // P              # tiles per batch = 8
BUCKET_ROWS = 32768           # int16 index reach
U_BUCKETS = math.ceil(N_USERS / BUCKET_ROWS)   # 4
V_BUCKETS = math.ceil(N_ITEMS / BUCKET_ROWS)   # 2
NJ = U_BUCKETS * V_BUCKETS                      # 8 joint buckets
CF = C * D                    # 640


def _default_nb():
    """Static batches-per-joint-bucket: mean + 6 sigma, rounded up to BATCH."""
    pu = [min(BUCKET_ROWS, N_USERS - i * BUCKET_ROWS) / N_USERS for i in range(U_BUCKETS)]
    pv = [min(BUCKET_ROWS, N_ITEMS - i * BUCKET_ROWS) / N_ITEMS for i in range(V_BUCKETS)]
    nb = []
    for iu in range(U_BUCKETS):
        for iv in range(V_BUCKETS):
            p = pu[iu] * pv[iv]
            mean = EL * p
            sig = math.sqrt(EL * p * (1 - p))
            nb.append(max(1, math.ceil((mean + 6 * sig + 1) / BATCH)))
    return tuple(nb)


DEFAULT_NB = _default_nb()


@functools.lru_cache(maxsize=4)
def _build_program(nb: tuple):
    nbtot = sum(nb)
    nidx_cols = nbtot * (BATCH // 16)  # int16 idx columns (wrapped in 16 partitions)

    nc = bacc.Bacc("TRN2", target_bir_lowering=False, debug=False,
                   num_devices=NCORES, num_swdge_queues=2)

    ut_d = nc.declare_dram_parameter("ut", [N_USERS, D], BF16, isOutput=False)
    it_d = nc.declare_dram_parameter("it", [N_ITEMS, D], BF16, isOutput=False)
    uidx_d = nc.declare_dram_parameter("uidx", [P, nidx_cols], I16, isOutput=False)
    vidx_d = nc.declare_dram_parameter("vidx", [P, nidx_cols], I16, isOutput=False)
    m2_d = nc.declare_dram_parameter("m2", [D, CF], BF16, isOutput=False)
    id_d = nc.declare_dram_parameter("ident", [P, P], BF16, isOutput=False)
    out_d = nc.declare_dram_parameter("outp", [P, nbtot, TPB * 8], F32, isOutput=True)

    # batch -> (user bucket base/rows, item bucket base/rows)
    batch_bk = []
    for jk, cnt in enumerate(nb):
        iu, iv = divmod(jk, V_BUCKETS)
        for _ in range(cnt):
            batch_bk.append((iu, iv))

    with TileContext(nc) as tc:
        with (
            tc.tile_pool(name="const", bufs=1) as const,
            tc.tile_pool(name="ug", bufs=4) as ugp,
            tc.tile_pool(name="vg", bufs=4) as vgp,
            tc.tile_pool(name="ut", bufs=6) as utp,
            tc.tile_pool(name="ysb", bufs=5) as ysbp,
            tc.tile_pool(name="prod", bufs=4) as prp,
            tc.tile_pool(name="stg", bufs=4) as stp,
            tc.tile_pool(name="psT", bufs=3, space="PSUM") as psT,
            tc.tile_pool(name="psY", bufs=2, space="PSUM") as psY,
        ):
            uidx_sb = const.tile([P, nidx_cols], I16)
            nc.sync.dma_start(out=uidx_sb[:], in_=uidx_d[:])
            vidx_sb = const.tile([P, nidx_cols], I16)
            nc.sync.dma_start(out=vidx_sb[:], in_=vidx_d[:])
            m2_sb = const.tile([D, CF], BF16)
            nc.sync.dma_start(out=m2_sb[:], in_=m2_d[:])
            id_sb = const.tile([P, P], BF16)
            nc.sync.dma_start(out=id_sb[:], in_=id_d[:])

            tc.strict_bb_all_engine_barrier()

            ic = BATCH // 16  # idx columns per batch
            for b in range(nbtot):
                iu, iv = batch_bk[b]
                ubase = iu * BUCKET_ROWS
                vbase = iv * BUCKET_ROWS

                ubuf = ugp.tile([P, TPB, D], BF16)
                nc.gpsimd.dma_gather(
                    out_ap=ubuf[:],
                    in_ap=ut_d[ubase:, :],
                    idxs_ap=uidx_sb[:, b * ic:(b + 1) * ic],
                    num_idxs=BATCH,
                    num_idxs_reg=BATCH,
                    elem_size=D,
                )
                vbuf = vgp.tile([P, TPB, D], BF16)
                nc.gpsimd.dma_gather(
                    out_ap=vbuf[:],
                    in_ap=it_d[vbase:, :],
                    idxs_ap=vidx_sb[:, b * ic:(b + 1) * ic],
                    num_idxs=BATCH,
                    num_idxs_reg=BATCH,
                    elem_size=D,
                    queue_num=1,
                )

                stage = stp.tile([P, TPB * 8], F32)
                nc.vector.memset(stage[:], 0.0)
                for t in range(TPB):
                    ups = psT.tile([P, P], BF16)
                    nc.tensor.transpose(out=ups[:], in_=ubuf[:, t, :],
                                        identity=id_sb[:])
                    uT = utp.tile([P, P], BF16)
                    # uT copies on ScalarE: VectorE (5x STT per tile) is the
                    # bottleneck engine, ScalarE has headroom
                    nc.scalar.copy(out=uT[:], in_=ups[:])

                    y_ps = psY.tile([P, CF], F32)
                    nc.tensor.matmul(out=y_ps[:, 0:512], lhsT=uT[:],
                                     rhs=m2_sb[:, 0:512], start=True, stop=True)
                    nc.tensor.matmul(out=y_ps[:, 512:CF], lhsT=uT[:],
                                     rhs=m2_sb[:, 512:CF], start=True, stop=True)

                    y_sb = ysbp.tile([P, CF], BF16)
                    nc.scalar.copy(out=y_sb[:], in_=y_ps[:])

                    prod = prp.tile([P, D], BF16)
                    for c in range(C):
                        nc.vector.scalar_tensor_tensor(
                            out=prod[:],
                            in0=y_sb[:, c * D:(c + 1) * D],
                            scalar=1.0,
                            in1=vbuf[:, t, :],
                            op0=mybir.AluOpType.mult,
                            op1=mybir.AluOpType.mult,
                            accum_out=stage[:, t * 8 + c:t * 8 + c + 1],
                        )
                # relu over the whole staging tile (cols 5..7 are garbage,
                # discarded by the host)
                nc.vector.tensor_scalar_max(out=stage[:], in0=stage[:], scalar1=0.0)
                nc.sync.dma_start(out=out_d[:, b, :], in_=stage[:])

    nc.compile()
    return nc, nbtot


def _prep_core(ui, vi, nb):
    """Bucket one core's edges; returns (u16, v16, orig_slot) arrays."""
    nbtot = sum(nb)
    nslots = nbtot * BATCH
    jb = (ui >> 15) * V_BUCKETS + (vi >> 15)
    # pad slots gather row 0 of their bucket (valid index; results discarded)
    u16 = np.zeros(nslots, np.int16)
    v16 = np.zeros(nslots, np.int16)
    orig = np.full(nslots, -1, np.int64)
    off = 0
    for jk in range(NJ):
        sel = np.nonzero(jb == jk)[0]
        cnt = len(sel)
        cap = nb[jk] * BATCH
        if cnt > cap:
            return None, None, None  # overflow -> caller enlarges nb
        iu, iv = divmod(jk, V_BUCKETS)
        u16[off:off + cnt] = (ui[sel] - iu * BUCKET_ROWS).astype(np.int16)
        v16[off:off + cnt] = (vi[sel] - iv * BUCKET_ROWS).astype(np.int16)
        orig[off:off + cnt] = sel
        off += cap
    return u16, v16, orig


def _wrap_idx(x16, nbtot):
    """[nslots] int16 -> [128, nbtot*64] SBUF layout (wrapped in 16
    partitions per batch, replicated 8x across partition groups)."""
    w = x16.reshape(nbtot, BATCH // 16, 16).transpose(2, 0, 1).reshape(16, -1)
    return np.ascontiguousarray(np.tile(w, (8, 1)))


def _prepare(user_inputs, item_inputs, user_indices, item_indices,
             weight, weight_classifier):
    user_inputs = np.asarray(user_inputs)
    item_inputs = np.asarray(item_inputs)
    ui_all = np.asarray(user_indices).astype(np.int64)
    vi_all = np.asarray(item_indices).astype(np.int64)
    weight = np.asarray(weight, dtype=np.float32)
    wc = np.asarray(weight_classifier, dtype=np.float32)

    # fold classifier into bilinear weights: M2[d, c*D+f]
    m2 = np.einsum("kdf,kc->cdf", weight, wc).transpose(1, 0, 2).reshape(D, CF)
    m2 = np.ascontiguousarray(m2).astype(bf16)
    ut_bf = np.ascontiguousarray(user_inputs.astype(bf16))
    it_bf = np.ascontiguousarray(item_inputs.astype(bf16))
    ident = np.eye(P, dtype=np.float32).astype(bf16)

    nb = DEFAULT_NB
    # per-core host prep (retry with enlarged buckets on overflow)
    while True:
        preps = []
        ok = True
        for c in range(NCORES):
            seg = slice(c * EL, (c + 1) * EL)
            u16, v16, orig = _prep_core(ui_all[seg], vi_all[seg], nb)
            if u16 is None:
                ok = False
                break
            preps.append((u16, v16, orig))
        if ok:
            break
        # enlarge: recompute from actual max counts across cores
        counts = np.zeros(NJ, np.int64)
        for c in range(NCORES):
            seg = slice(c * EL, (c + 1) * EL)
            jb = (ui_all[seg] >> 15) * V_BUCKETS + (vi_all[seg] >> 15)
            counts = np.maximum(counts, np.bincount(jb, minlength=NJ))
        nb = tuple(int(math.ceil((cn + 1) / BATCH)) for cn in counts)

    nc, nbtot = _build_program(nb)

    in_maps = []
    for c in range(NCORES):
        u16, v16, orig = preps[c]
        in_maps.append({
            "ut": ut_bf,
            "it": it_bf,
            "uidx": _wrap_idx(u16, nbtot),
            "vidx": _wrap_idx(v16, nbtot),
            "m2": m2,
            "ident": ident,
        })

    return nc, nbtot, in_maps, preps


def _postprocess(results, nbtot, preps):
    out = np.empty((E, C), np.float32)
    for c in range(NCORES):
        o = results[c]["outp"]  # [128, nbtot, 64]
        slotted = o.reshape(P, nbtot, TPB, 8).transpose(1, 2, 0, 3).reshape(-1, 8)
        _, _, orig = preps[c]
        mask = orig >= 0
        out[c * EL + orig[mask]] = slotted[mask][:, :C]
    return out


def kernel(user_inputs, item_inputs, user_indices, item_indices,
           weight, weight_classifier):
    nc, nbtot, in_maps, preps = _prepare(
        user_inputs, item_inputs, user_indices, item_indices,
        weight, weight_classifier)
    results = run_bass_kernel_spmd(nc, in_maps, list(range(NCORES))).results
    return _postprocess(results, nbtot, preps)


def timed_run(inputs):
    """Run with tracing; returns HW exec time in ns (or None)."""
    nc, nbtot, in_maps, preps = _prepare(**inputs)
    r = run_bass_kernel_spmd(nc, in_maps, list(range(NCORES)), trace=True)
    if r.profile_json:
        print("profile:", r.profile_json)
    return r.exec_time_ns

